# revision 2
# baseline (speedup 1.0000x reference)
"""Trainium2 Bass kernel for nn_CVXPolicy_Integrator (v3, bf16).

Computation (per sample):
    h = [t, z]                      # [257]
    p = tanh(h @ W1 + b1) @ W2 + b2 # [256]
    r2 = ||p||^2
    w  = LambertW(r2) via Newton
    ustar = -sqrt(w / r2) * p       (scale -> 1 as r2 -> 0)

Strategy: pure data parallel over batch B=131072 across 8 cores
(16384 rows/core), bf16 end-to-end (rel-err gate 2e-2; measured
~4e-3).  Structure per core (32 super-tiles of 512 samples, 2
newton halves):

  - All matmuls bf16 (1 cyc/row on PE); inputs shipped feature-major
    bf16, outputs returned bf16.
  - r2 never touches p: with M = W2aug W2aug^T = L L^T (host
    Cholesky), r2_j = ||L^T s_j||^2.  v = L^T s is one matmul, v^2
    runs on ACT (Square -> bf16), and the 101-row column sum is a
    shifted-mask matmul accumulating super-tile j's sums into row j
    of a per-half [16,512] PSUM strip; PE transposes deliver the
    [128,64] layout the Newton solve wants.
  - s stays resident in SBUF; the single L2 pass runs in phase 3
    where the per-sample scale is fused into the PSUM->SBUF drain
    (ACT Copy with per-partition AP scale / DVE tensor_tensor with a
    free-broadcast scale), so p is never stored unscaled.  GPSIMD
    cannot read PSUM and its ALU ops are ~4us each on hardware, so
    Pool only issues the output DMAs (software DGE queue - also
    spreads DMA across a second queue, inputs use the SP + ACT HWDGE
    queues).
  - Newton (4 iters, w' = (w^2 + r2 e^-w)/(1+w)) seeds from a
    bit-hack log2 and finishes with a bit-hack rsqrt: only
    Exp/Tanh/Square/Copy activation funcs -> one table set, zero
    reloads.
  - Emission is software-pipelined: v/square/colsum lag L1 by one
    super-tile so the PE never waits on tanh; phase A of half h+1 is
    interleaved with phase 3 of half h across the newton barrier.
"""

import sys

import numpy as np

sys.path.insert(0, "/opt/trn_rl_repo")

import concourse.bacc as bacc  # noqa: E402
import concourse.bass as bass  # noqa: E402
import concourse.mybir as mybir  # noqa: E402
import concourse.tile as tile  # noqa: E402
from concourse import bass_utils  # noqa: E402

import ml_dtypes  # noqa: E402

F32 = mybir.dt.float32
BF16 = mybir.dt.bfloat16
I32 = mybir.dt.int32
AF = mybir.ActivationFunctionType
ALU = mybir.AluOpType
NPBF16 = ml_dtypes.bfloat16

B, D, H = 131072, 256, 100
HP = H + 1  # augmented hidden (bias row)
NCORES = 8
BPC = B // NCORES  # 16384 rows per core
ST = 512  # samples per super-tile
NST = BPC // ST  # 32 super-tiles
NH = 4  # newton batches
HST = NST // NH  # super-tiles per half
NEWTON_ITERS = 4

LN2_2P23 = float(np.log(2.0) / (1 << 23))
EXP_BIAS = 127 << 23  # 1065353216


def build_nc(bpc: int = BPC, compile_bacc: bool = True) -> bass.Bass:
    nst = bpc // ST
    hst = nst // NH

    nc = bacc.Bacc("TRN2")

    hT = nc.dram_tensor("hT", [D + 1, bpc], BF16, kind="ExternalInput")
    w1a_d = nc.dram_tensor("w1a", [128, H], BF16, kind="ExternalInput")
    w1b_d = nc.dram_tensor("w1b", [128, H], BF16, kind="ExternalInput")
    w1t_d = nc.dram_tensor("w1t", [1, H], BF16, kind="ExternalInput")
    rl_d = nc.dram_tensor("rl", [HP, HP], BF16, kind="ExternalInput")
    w2_d = nc.dram_tensor("w2a", [HP, D], BF16, kind="ExternalInput")
    tmask_d = nc.dram_tensor("tmask", [HP, 3 * HST], BF16, kind="ExternalInput")
    ident_d = nc.dram_tensor("ident", [HST, HST], F32, kind="ExternalInput")
    b1_d = nc.dram_tensor("b1c", [H, 1], F32, kind="ExternalInput")
    ones_d = nc.dram_tensor("onesrow", [1, bpc], BF16, kind="ExternalInput")
    out_d = nc.dram_tensor("out", [bpc, D], BF16, kind="ExternalOutput")

    with tile.TileContext(nc) as tc:
        with (
            tc.tile_pool(name="const", bufs=1) as const,
            tc.tile_pool(name="sall", bufs=1) as sallp,
            tc.tile_pool(name="zp", bufs=3) as zp,
            tc.tile_pool(name="tp", bufs=2) as tp,
            tc.tile_pool(name="pp", bufs=3) as pp,
            tc.tile_pool(name="up", bufs=3) as up,
            tc.tile_pool(name="rsp", bufs=2) as rsp,
            tc.tile_pool(name="nt", bufs=2) as nt,
            tc.tile_pool(name="scp", bufs=2) as scp,
            tc.tile_pool(name="aps", bufs=2, space="PSUM") as aps,
            tc.tile_pool(name="vps", bufs=2, space="PSUM") as vps,
            tc.tile_pool(name="raccp", bufs=1, space="PSUM") as raccp,
            tc.tile_pool(name="r2tp", bufs=1, space="PSUM") as r2tp,
            tc.tile_pool(name="pps", bufs=2, space="PSUM") as pps,
        ):
            w1a = const.tile([128, H], BF16)
            nc.sync.dma_start(w1a[:], w1a_d[:])
            w1b = const.tile([128, H], BF16)
            nc.sync.dma_start(w1b[:], w1b_d[:])
            w1t = const.tile([1, H], BF16)
            nc.sync.dma_start(w1t[:], w1t_d[:])
            rl = const.tile([HP, HP], BF16)
            nc.sync.dma_start(rl[:], rl_d[:])
            w2a = const.tile([HP, D], BF16)
            nc.sync.dma_start(w2a[:], w2_d[:])
            tmask = const.tile([HP, 3 * HST], BF16)
            nc.sync.dma_start(tmask[:], tmask_d[:])
            ident = const.tile([HST, HST], F32)
            nc.sync.dma_start(ident[:], ident_d[:])
            b1c = const.tile([H, 1], F32)
            nc.sync.dma_start(b1c[:], b1_d[:])

            # s_all: tanh activations, resident.  Row 100 = 1.0 (bias unit),
            # loaded once from DRAM; tanh only ever writes rows 0:100.
            s_all = sallp.tile([128, nst * ST], BF16)
            nc.sync.dma_start(s_all[H : H + 1, :], ones_d[:])

            # r2T: transposed r2; half h occupies columns [h*4*HST, (h+1)*4*HST).
            r2T = r2tp.tile([128, NH * 4 * HST], F32, name="r2T")

            racc_tiles = {}

            def load_st(h: int, j: int, state: dict):
                st = h * hst + j
                c0 = st * ST
                if state.get("last_loaded") == st:
                    return
                state["last_loaded"] = st
                if j % 2 == 0:
                    zab = zp.tile([128, 4 * ST], BF16, tag="zab")
                    # alternate the two HWDGE queues; start on scalar so the
                    # first z tiles don't queue behind the const loads on sync
                    eng = nc.scalar if (st // 2) % 2 == 0 else nc.sync
                    eng.dma_start(zab[:, 0 : 2 * ST], hT[0:128, c0 : c0 + 2 * ST])
                    eng.dma_start(
                        zab[:, 2 * ST : 4 * ST], hT[128:256, c0 : c0 + 2 * ST]
                    )
                    state["zab"] = zab
                if j % 8 == 0:
                    tr8 = tp.tile([1, 8 * ST], BF16, tag="tr")
                    nc.sync.dma_start(tr8[:], hT[D : D + 1, c0 : c0 + 8 * ST])
                    state["tr8"] = tr8

            def l1_st(h: int, j: int, state: dict):
                st = h * hst + j
                zab = state["zab"]
                tr8 = state["tr8"]
                jj = j % 2
                zA = zab[:, jj * ST : (jj + 1) * ST]
                zB = zab[:, 2 * ST + jj * ST : 2 * ST + (jj + 1) * ST]
                tR = tr8[:, (j % 8) * ST : (j % 8 + 1) * ST]

                a_ps = aps.tile([128, ST], F32, tag="a")
                nc.tensor.matmul(a_ps[0:H, :], w1a[:], zA, start=True, stop=False)
                nc.tensor.matmul(a_ps[0:H, :], w1b[:], zB, start=False, stop=False)
                nc.tensor.matmul(a_ps[0:H, :], w1t[:], tR, start=False, stop=True)

                scol = s_all[:, st * ST : (st + 1) * ST]
                nc.scalar.activation(scol[0:H, :], a_ps[0:H, :], AF.Tanh, bias=b1c[:])

            def vcs_st(h: int, j: int):
                # v / square / colsum for super-tile j of half h (lags L1
                # by one super-tile so the PE never waits on tanh).
                st = h * hst + j
                scol = s_all[:, st * ST : (st + 1) * ST]
                v_ps = vps.tile([128, ST], F32, tag="v")
                nc.tensor.matmul(
                    v_ps[0:HP, :], rl[:], scol[0:HP, :], start=True, stop=True
                )
                prod = pp.tile([128, ST], BF16, tag="prod")
                nc.scalar.activation(prod[0:HP, :], v_ps[0:HP, :], AF.Square)
                if j == 0:
                    racc_tiles[h] = raccp.tile(
                        [2 * HST, ST], F32, tag="racc", name=f"racc_{h}"
                    )
                nc.tensor.matmul(
                    racc_tiles[h][:, :],
                    tmask[:, HST - j : 3 * HST - j],
                    prod[0:HP, :],
                    start=(j == 0),
                    stop=(j == hst - 1),
                )

            def r2fin(h: int):
                racc_sb = rsp.tile([HST, ST], F32, tag="raccsb")
                nc.vector.tensor_copy(racc_sb[:], racc_tiles[h][0:HST, :])
                for k in range(ST // 128):
                    nc.tensor.transpose(
                        r2T[:, h * 4 * HST + k * HST : h * 4 * HST + (k + 1) * HST],
                        racc_sb[:, k * 128 : (k + 1) * 128],
                        ident[:],
                    )

            def newton(h: int):
                wd = HST * 4

                def tmp(tag, dtype=F32):
                    return nt.tile([128, wd], dtype, tag=tag, name=f"nt_{tag}_{h}")

                r2 = tmp("r2")
                nc.vector.tensor_copy(r2[:], r2T[:, h * wd : (h + 1) * wd])

                # seed: w0 = ln2 * log2(1 + r2) via float bit hack
                x = tmp("x")
                nc.vector.tensor_scalar_add(x[:], r2[:], 1.0)
                xi = tmp("xi", I32)
                nc.vector.tensor_scalar_sub(xi[:], x[:].bitcast(I32), EXP_BIAS)
                w = tmp("w")
                nc.vector.tensor_scalar_mul(w[:], xi[:], LN2_2P23)

                for _ in range(NEWTON_ITERS):
                    # e on ACT first; independent DVE ops overlap its latency
                    e = tmp("e")
                    nc.scalar.activation(e[:], w[:], AF.Exp, scale=-1.0)
                    b1t = tmp("b1t")
                    nc.vector.scalar_tensor_tensor(
                        b1t[:], w[:], 1.0, w[:], op0=ALU.mult, op1=ALU.mult
                    )
                    d1 = tmp("d1")
                    nc.vector.tensor_scalar_add(d1[:], w[:], 1.0)
                    rd = tmp("rd")
                    nc.vector.reciprocal(rd[:], d1[:])
                    a1 = tmp("a1")
                    nc.vector.tensor_mul(a1[:], r2[:], e[:])
                    c1 = tmp("c1")
                    nc.vector.tensor_add(c1[:], a1[:], b1t[:])
                    wn = tmp("w")
                    nc.vector.tensor_mul(wn[:], c1[:], rd[:])
                    w = wn

                # scale = -sqrt(w / r2)  (tends to -1 as r2 -> 0)
                rg = tmp("rg")
                nc.vector.tensor_scalar_max(rg[:], r2[:], 1e-30)
                rr = tmp("rr")
                nc.vector.reciprocal(rr[:], rg[:])
                q = tmp("q")
                nc.vector.tensor_mul(q[:], w[:], rr[:])
                # rsqrt bit hack + 2 Newton iterations
                qi = tmp("qi", I32)
                nc.vector.tensor_scalar(
                    qi[:], q[:].bitcast(I32), 1, None, op0=ALU.logical_shift_right
                )
                yi = tmp("yi", I32)
                nc.vector.tensor_scalar(
                    yi[:], qi[:], -1, 0x5F3759DF, op0=ALU.mult, op1=ALU.add
                )
                y = yi[:].bitcast(F32)
                for it in range(2):
                    t2 = tmp(f"t2_{it}")
                    nc.vector.tensor_mul(t2[:], y, q[:])
                    u2_ = tmp(f"u2_{it}")
                    nc.vector.tensor_mul(u2_[:], t2[:], y)
                    v2 = tmp(f"v2_{it}")
                    nc.vector.tensor_scalar(
                        v2[:], u2_[:], -0.5, 1.5, op0=ALU.mult, op1=ALU.add
                    )
                    yn = tmp(f"y_{it}")
                    nc.vector.tensor_mul(yn[:], y, v2[:])
                    y = yn[:]
                sc = scp.tile([128, wd], F32, tag="sc", name=f"sc_{h}")
                nc.vector.scalar_tensor_tensor(
                    sc[:], q[:], -1.0, y, op0=ALU.mult, op1=ALU.mult
                )
                return sc

            def phase3_st(h: int, j: int, sc, state: dict):
                st = h * hst + j
                jj = j % 2
                if jj == 0:
                    state["u2"] = up.tile(
                        [128, 4 * ST], BF16, tag="u", name=f"u2_{h}_{j}"
                    )
                u2 = state["u2"]
                scol = s_all[:, st * ST : (st + 1) * ST]
                for half_k in range(2):
                    p_ps = pps.tile([128, ST], F32, tag="p")
                    for k2 in range(2):
                        k = half_k * 2 + k2
                        nc.tensor.matmul(
                            p_ps[:, k2 * D : (k2 + 1) * D],
                            scol[0:HP, k * 128 : (k + 1) * 128],
                            w2a[:],
                            start=True,
                            stop=True,
                        )
                    k0 = half_k * 2
                    col0 = k0 * HST + j
                    if (2 * j + half_k) % 8 < 2:
                        # ACT share: two [128,256] Copy-with-scale ops
                        for k2 in range(2):
                            k = k0 + k2
                            nc.scalar.activation(
                                u2[:, (jj * 4 + k) * D : (jj * 4 + k + 1) * D],
                                p_ps[:, k2 * D : (k2 + 1) * D],
                                AF.Copy,
                                scale=sc[:, k * HST + j : k * HST + j + 1],
                            )
                    else:
                        # DVE: one [128,512] drain; scale strided over the two
                        # groups (cols col0, col0+HST) broadcast along D
                        sc2 = (
                            sc[:, col0 : col0 + HST + 1 : HST]
                            .unsqueeze(-1)
                            .broadcast_to([128, 2, D])
                        )
                        nc.vector.tensor_mul(
                            u2[:, (jj * 4 + k0) * D : (jj * 4 + k0 + 2) * D].rearrange(
                                "p (g d) -> p g d", g=2
                            ),
                            p_ps[:].rearrange("p (g d) -> p g d", g=2),
                            sc2,
                        )
                if jj == 1:
                    r0 = (st - 1) * ST
                    nc.gpsimd.dma_start(
                        out_d[r0 : r0 + 2 * ST, :].rearrange(
                            "(j2 k p) d -> p j2 k d", j2=2, k=4
                        ),
                        u2[:].rearrange("p (j2 k d) -> p j2 k d", j2=2, k=4),
                    )

            # ---- emission schedule ----
            stA: dict = {}
            st3: dict = {}
            scs: dict = {}

            load_st(0, 0, stA)  # prefetch ahead of the const DMAs
            for j in range(hst):
                load_st(0, j, stA)
                if j > 0:
                    vcs_st(0, j - 1)
                l1_st(0, j, stA)
            vcs_st(0, hst - 1)
            r2fin(0)
            scs[0] = newton(0)
            for h in range(1, NH):
                for j in range(hst):
                    load_st(h, j, stA)
                    if j > 0:
                        vcs_st(h, j - 1)
                    l1_st(h, j, stA)
                    phase3_st(h - 1, j, scs[h - 1], st3)
                vcs_st(h, hst - 1)
                r2fin(h)
                scs[h] = newton(h)
            for j in range(hst):
                phase3_st(NH - 1, j, scs[NH - 1], st3)

    if compile_bacc:
        nc.compile()
    return nc


_NC_CACHE: dict[int, bass.Bass] = {}


def _get_nc(bpc: int) -> bass.Bass:
    if bpc not in _NC_CACHE:
        _NC_CACHE[bpc] = build_nc(bpc)
    return _NC_CACHE[bpc]


def make_in_maps(z, t, W1, b1, W2, b2, ncores=NCORES):
    z = np.ascontiguousarray(np.asarray(z, dtype=np.float32))
    t = np.ascontiguousarray(np.asarray(t, dtype=np.float32))
    W1 = np.asarray(W1, dtype=np.float32)
    b1 = np.asarray(b1, dtype=np.float32)
    W2 = np.asarray(W2, dtype=np.float32)
    b2 = np.asarray(b2, dtype=np.float32)
    bpc = z.shape[0] // ncores

    w1a = W1[1:129].astype(NPBF16)
    w1b = W1[129:257].astype(NPBF16)
    w1t = W1[0:1].astype(NPBF16)
    w2aug = np.concatenate([W2, b2[None, :]], axis=0)  # [101, 256]
    m = w2aug.astype(np.float64) @ w2aug.astype(np.float64).T
    m[np.diag_indices_from(m)] += 1e-9
    rl = np.linalg.cholesky(m)  # lower: m = rl @ rl.T; v = rl.T @ s
    rl_b = rl.astype(NPBF16)
    w2a = w2aug.astype(NPBF16)
    tmask = np.zeros((HP, 3 * (NST // NH)), dtype=NPBF16)
    tmask[:, NST // NH] = NPBF16(1.0)
    ident = np.eye(NST // NH, dtype=np.float32)
    b1c = np.ascontiguousarray(b1[:, None], dtype=np.float32)
    onesrow = np.ones((1, bpc), dtype=NPBF16)

    zb = z.astype(NPBF16)
    tb = t.astype(NPBF16)
    in_maps = []
    for c in range(ncores):
        sl = slice(c * bpc, (c + 1) * bpc)
        hTc = np.empty((D + 1, bpc), NPBF16)
        hTc[:D] = zb[sl].T
        hTc[D] = tb[sl, 0]
        in_maps.append(
            {
                "hT": hTc,
                "w1a": w1a,
                "w1b": w1b,
                "w1t": w1t,
                "rl": rl_b,
                "w2a": w2a,
                "tmask": tmask,
                "ident": ident,
                "b1c": b1c,
                "onesrow": onesrow,
            }
        )
    return in_maps


def kernel(z, t, W1, b1, W2, b2):
    in_maps = make_in_maps(z, t, W1, b1, W2, b2)
    nc = _get_nc(BPC)
    res = bass_utils.run_bass_kernel_spmd(nc, in_maps, list(range(NCORES))).results
    return np.concatenate(
        [np.asarray(res[c]["out"]).astype(np.float32) for c in range(NCORES)], axis=0
    )
